# revision 17
# baseline (speedup 1.0000x reference)
"""Trainium2 Bass kernel for nn_DCTiDCTWrapper3D.

Math: out = x + gelu(conv1x1(irfft(rfft(x, ch-axis) * Wc, ch-axis)) + b)

The rfft -> complex-filter -> irfft chain along the 1024-channel axis is a
linear map, implemented as two dense 1024x1024 matmuls with a packed real
spectrum of exactly 1024 slots (513 Re + 511 Im; the imaginary parts of bins
0 and 512 never reach the irfft output). The irfft basis is folded into the
1x1 conv weights host-side (M2 = Gp @ conv_w.T), so the device computes:

  per (b, t):  XF^T = M1^T @ x          [1024 slots, 392 pos]  (PE)
               Y_c  = XF_c*A_c + XF_{(c+4)%8}*B_c              (DVE)
               e^T  = gelu(M2^T @ Y + b) [1024 cout, 392 pos]  (PE+ACT)

and ships e back in bf16; the residual out = x + e happens on the host
(the residual path involves no device compute and skipping it removes
25 MB/core of fp32 DMA).

Device pipeline notes (from perfetto trace analysis of v1):
- The PE streams fp8 DoubleRow matmuls at ~163ns each (157 TF/s) with
  LDWEIGHTS fully hidden, so 512 matmuls/core = ~84us is the PE floor.
  Everything else is sized to hide underneath it.
- DVE fast modes need all operands 2-byte, packed, in SBUF: the spectral
  filter A/B are pre-tiled per position host-side (b-duplicated) so the
  muls are plain elementwise bf16, no stride-0 broadcast.
- PSUM->SBUF X copies are pair-fused on ACT: stage-1 writes chunk pairs
  (c, c+4) into one 2-bank PSUM tile, one ACT copy moves both (the two
  chunks a filter pair needs), halving ACT's ~185ns fixed per-op cost.
- One contiguous DMA per tensor per t (128 descriptors of 3-6KB each vs
  53k small descriptors in v1).

Sharding: data-parallel over the batch dim b (16 clips / 8 cores = 2 per
core); filter + DFT/conv weights replicated.
"""

import os
import sys

import numpy as np

for _p in ("/opt/trn_rl_repo", "/root/.axon_site/_ro/trn_rl_repo"):
    if os.path.isdir(_p) and _p not in sys.path:
        sys.path.append(_p)

import ml_dtypes

import concourse.bass as bass
import concourse.mybir as mybir
import concourse.tile as tile
from concourse import bacc
from concourse.bass_utils import run_bass_kernel_spmd

B, T, C, H, W = 16, 8, 1024, 14, 14
HW = H * W            # 196
NCORES = 8
BPC = B // NCORES     # 2 samples per core
KCH = C // 128        # 8 channel/slot chunks
NPOS = BPC * HW       # 392 matmul free dim
NPAD = 400            # fp8 DoubleRow interleave stride (16B aligned)
F32 = mybir.dt.float32
BF16 = mybir.dt.bfloat16
FP8 = mybir.dt.float8e4
NP_FP8 = ml_dtypes.float8_e4m3
NP_BF16 = ml_dtypes.bfloat16
DR = mybir.MatmulPerfMode.DoubleRow
PF = 2                # DMA prefetch depth in t iterations


def _dft_matrices():
    """Packed rfft matrix M1 [c, slot] and packed irfft basis Gp [slot, c],
    both ortho-normalized, slot layout: 0..512 Re(bin), 512+k Im(bin k)."""
    n = np.arange(C, dtype=np.float64)
    k_re = np.arange(513, dtype=np.float64)
    k_im = np.arange(1, 512, dtype=np.float64)
    s = 1.0 / np.sqrt(C)
    M1 = np.empty((C, C), np.float64)
    M1[:, :513] = np.cos(2 * np.pi * np.outer(n, k_re) / C) * s
    M1[:, 513:] = -np.sin(2 * np.pi * np.outer(n, k_im) / C) * s
    Gp = np.empty((C, C), np.float64)
    Gp[0, :] = s
    Gp[1:512, :] = 2.0 * np.cos(2 * np.pi * np.outer(k_im, n) / C) * s
    Gp[512, :] = ((-1.0) ** n) * s
    Gp[513:, :] = -2.0 * np.sin(2 * np.pi * np.outer(k_im, n) / C) * s
    return M1, Gp


def _pack_weights_fp8(Mx):
    """[c, slot] -> [128, mf, g, i, 128] fp8 flat [128, 8192] for DoubleRow
    lhsT tiles (c = g*256 + i*128 + p), output-chunk-major so the weight
    load can be split into per-chunk DMAs at startup."""
    w = Mx.astype(np.float32).reshape(4, 2, 128, KCH, 128)  # g i p mf c
    w = w.transpose(2, 3, 0, 1, 4).reshape(128, 4 * 2 * KCH * 128)
    return np.ascontiguousarray(w).astype(NP_FP8)


def _build_nc():
    nc = bacc.Bacc(
        "TRN2", target_bir_lowering=False, debug=False, num_devices=NCORES
    )
    x8_d = nc.dram_tensor("x8", [T, 128, 4, 2, NPAD], FP8, kind="ExternalInput").ap()
    m1_d = nc.dram_tensor("m1", [128, KCH, 4 * 2 * 128], FP8, kind="ExternalInput").ap()
    m2_d = nc.dram_tensor("m2", [128, KCH * C], FP8, kind="ExternalInput").ap()
    ab_d = nc.dram_tensor("ab", [T, 128, 2, KCH, NPOS], BF16, kind="ExternalInput").ap()
    bias_d = nc.dram_tensor("bias", [128, KCH], F32, kind="ExternalInput").ap()
    out_d = nc.dram_tensor("out", [T, 128, KCH, NPOS], BF16, kind="ExternalOutput").ap()

    from contextlib import ExitStack

    with tile.TileContext(nc) as tc, ExitStack() as ctx:
        const = ctx.enter_context(tc.tile_pool(name="const", bufs=1))
        x8_pool = ctx.enter_context(tc.tile_pool(name="x8", bufs=PF + 1))
        ab_pool = ctx.enter_context(tc.tile_pool(name="ab", bufs=PF + 1))
        xb_pool = ctx.enter_context(tc.tile_pool(name="xb", bufs=2))
        y_pool = ctx.enter_context(tc.tile_pool(name="y", bufs=2))
        tmp_pool = ctx.enter_context(tc.tile_pool(name="tmp", bufs=4))
        o_pool = ctx.enter_context(tc.tile_pool(name="o", bufs=3))
        ps1_pool = ctx.enter_context(tc.tile_pool(name="ps1", bufs=2, space="PSUM"))
        ps2_pool = ctx.enter_context(tc.tile_pool(name="ps2", bufs=3, space="PSUM"))

        m1_sb = const.tile([128, KCH, 4, 2, 128], FP8)
        m2_sb = const.tile([128, KCH * C], FP8)
        bias_sb = const.tile([128, KCH], F32)
        m2_t = m2_sb[:, :].rearrange("p (m g i c) -> p m g i c", g=4, i=2, m=KCH)

        x8_tiles, ab_tiles = {}, {}

        def issue_dma(t):
            x8t = x8_pool.tile([128, 4, 2, NPAD], FP8, tag="x8")
            nc.sync.dma_start(x8t[:, :, :, :], x8_d[t])
            ab_sb = ab_pool.tile([128, 2, KCH, NPOS], BF16, tag="ab")
            nc.sync.dma_start(ab_sb[:, :, :, :], ab_d[t])
            x8_tiles[t], ab_tiles[t] = x8t, ab_sb

        # startup order: what the first stage-1 pair needs lands first —
        # x8 of t0 and the m1 chunks for pair 0 (m1 is packed host-side in
        # pair order 0,4,1,5,2,6,3,7 so this is a 2-chunk prefix) — then
        # the rest of m1, the t0/t1 filters, and the stage-2 weights
        x80 = x8_pool.tile([128, 4, 2, NPAD], FP8, tag="x8")
        nc.sync.dma_start(x80[:, :, :, :], x8_d[0])
        nc.sync.dma_start(m1_sb[:, 0:2], m1_d[:, 0:2])
        ab0 = ab_pool.tile([128, 2, KCH, NPOS], BF16, tag="ab")
        nc.sync.dma_start(ab0[:, :, :, :], ab_d[0])
        x8_tiles[0], ab_tiles[0] = x80, ab0
        x81 = x8_pool.tile([128, 4, 2, NPAD], FP8, tag="x8")
        nc.sync.dma_start(x81[:, :, :, :], x8_d[1])
        nc.sync.dma_start(m1_sb[:, 2:8], m1_d[:, 2:8])
        nc.sync.dma_start(bias_sb[:, :], bias_d[:, :])
        ab1 = ab_pool.tile([128, 2, KCH, NPOS], BF16, tag="ab")
        nc.sync.dma_start(ab1[:, :, :, :], ab_d[1])
        x8_tiles[1], ab_tiles[1] = x81, ab1
        nc.sync.dma_start(m2_sb[:, :], m2_d[:, :])

        def stage2(t, y_sb):
            o_t = o_pool.tile([128, KCH, NPOS], BF16, tag="o")
            for mo in range(KCH):
                ps = ps2_pool.tile([128, NPOS], F32, tag="ps2")
                for g in range(4):
                    nc.tensor.matmul(
                        ps[:, :],
                        m2_t[:, mo, g, :, :],
                        y_sb[:, g, :, :NPOS],
                        start=(g == 0),
                        stop=(g == 3),
                        perf_mode=DR,
                    )
                nc.scalar.activation(
                    o_t[:, mo, :],
                    ps[:, :],
                    mybir.ActivationFunctionType.Gelu,
                    bias=bias_sb[:, mo : mo + 1],
                )
                # drain the first half early so the tail t's output DMA
                # overlaps the second half's compute
                if mo == 3:
                    nc.sync.dma_start(out_d[t][:, :4, :], o_t[:, :4, :])
            nc.sync.dma_start(out_d[t][:, 4:, :], o_t[:, 4:, :])

        pending = None
        for t in range(T):
            if t + PF < T:
                issue_dma(t + PF)
            x8t, ab_sb = x8_tiles.pop(t), ab_tiles.pop(t)

            # stage 1: X = M1^T @ x, chunk pairs (c, c+4) share a 2-bank
            # PSUM tile so one ACT copy drains both
            xb_t = xb_pool.tile([128, 4, 2, NPOS], BF16, tag="xb")
            y_sb = y_pool.tile([128, 4, 2, NPAD], FP8, tag="y")
            for p in range(4):
                ps = ps1_pool.tile([128, 2, 512], F32, tag="ps1")
                for half, c in ((0, p), (1, p + 4)):
                    for g in range(4):
                        nc.tensor.matmul(
                            ps[:, half, :NPOS],
                            m1_sb[:, 2 * (c % 4) + c // 4, g, :, :],
                            x8t[:, g, :, :NPOS],
                            start=(g == 0),
                            stop=(g == 3),
                            perf_mode=DR,
                        )
                nc.scalar.copy(xb_t[:, p, :, :], ps[:, :, :NPOS])
                # spectral filter: Y_c = A_c*X_c + B_c*X_{c+4} (and mirror)
                for half, c in ((0, p), (1, p + 4)):
                    p_t = tmp_pool.tile([128, NPOS], BF16, tag="p")
                    q_t = tmp_pool.tile([128, NPOS], BF16, tag="q")
                    nc.vector.tensor_mul(p_t[:, :], xb_t[:, p, half, :], ab_sb[:, 0, c, :])
                    nc.vector.tensor_mul(q_t[:, :], xb_t[:, p, 1 - half, :], ab_sb[:, 1, c, :])
                    nc.vector.tensor_add(y_sb[:, c // 2, c % 2, :NPOS], p_t[:, :], q_t[:, :])

            # stage 2 runs one t behind stage 1 so the PE stream
            # S1(t) -> S2(t-1) -> S1(t+1) never waits on the filter chain
            if pending is not None:
                stage2(*pending)
            pending = (t, y_sb)
        stage2(*pending)

    nc.compile()
    return nc


_CACHE = {}


def _get_compiled():
    if "nc" not in _CACHE:
        _CACHE["nc"] = _build_nc()
    return _CACHE["nc"]


def _host_prep(wfilt, conv_w, conv_b):
    M1, Gp = _dft_matrices()
    M2 = Gp @ conv_w.astype(np.float64).T
    m1_np = _pack_weights_fp8(M1)
    m2_np = _pack_weights_fp8(M2)

    # A/B filter tensors: [t, slot, hw], slots packed as in _dft_matrices
    Wr = wfilt[..., 0].reshape(T, HW, 513).transpose(0, 2, 1)  # [t, k, hw]
    Wi = wfilt[..., 1].reshape(T, HW, 513).transpose(0, 2, 1)
    A = np.zeros((T, C, HW), np.float32)
    Bf = np.zeros((T, C, HW), np.float32)
    A[:, :513] = Wr
    A[:, 513:] = Wr[:, 1:512]
    Bf[:, 1:512] = -Wi[:, 1:512]
    Bf[:, 513:] = Wi[:, 1:512]
    # chunk to [t, 128, k, hw], duplicate positions for the BPC=2 samples,
    # and stack A/B into one tensor (one DMA per t)
    A = A.reshape(T, KCH, 128, HW).transpose(0, 2, 1, 3)
    Bf = Bf.reshape(T, KCH, 128, HW).transpose(0, 2, 1, 3)
    ab = np.stack([A, Bf], axis=1)  # [t, 2, 128, k, hw]
    ab = np.tile(ab.transpose(0, 2, 1, 3, 4), (1, 1, 1, 1, BPC))
    ab_np = np.ascontiguousarray(ab).astype(NP_BF16)
    # [128, KCH, g*i*128] with chunks in filter-pair order (0,4,1,5,...) so
    # the first startup DMA carries exactly the two chunks pair 0 needs
    m1_np = np.ascontiguousarray(
        m1_np.reshape(128, KCH, 1024)[:, [0, 4, 1, 5, 2, 6, 3, 7], :]
    )
    bias_np = np.ascontiguousarray(conv_b.reshape(KCH, 128).T.astype(np.float32))
    return m1_np, m2_np, ab_np, bias_np


def _run(x, wfilt, conv_w, conv_b, n_segments, **spmd_kwargs):
    assert int(n_segments) == T and x.shape == (B * T, C, H, W)
    x = np.ascontiguousarray(x, dtype=np.float32)
    m1_np, m2_np, ab_np, bias_np = _host_prep(
        np.asarray(wfilt, np.float32),
        np.asarray(conv_w, np.float32),
        np.asarray(conv_b, np.float32),
    )
    # x [bt, c, h, w] -> fp8 [core, t, 128, g, i, NPAD] with both local
    # samples adjacent on the position axis (b-major within a t)
    x8 = x.reshape(NCORES, BPC, T, 4, 2, 128, HW).astype(NP_FP8)
    x8 = x8.transpose(0, 2, 5, 3, 4, 1, 6)  # core t p g i b s
    x8p = np.zeros((NCORES, T, 128, 4, 2, NPAD), NP_FP8)
    x8p[..., :NPOS] = x8.reshape(NCORES, T, 128, 4, 2, NPOS)

    nc = _get_compiled()
    in_maps = [
        {
            "x8": x8p[i],
            "m1": m1_np,
            "m2": m2_np,
            "ab": ab_np,
            "bias": bias_np,
        }
        for i in range(NCORES)
    ]
    res = run_bass_kernel_spmd(nc, in_maps, list(range(NCORES)), **spmd_kwargs)
    e = np.stack([r["out"] for r in res.results])  # [core, t, 128, k, 392]
    e = e.reshape(NCORES, T, 128, KCH, BPC, HW).astype(np.float32)
    e = e.transpose(0, 4, 1, 3, 2, 5).reshape(B * T, C, H, W)
    full = x + e
    return full, res


def kernel(x, wfilt, conv_w, conv_b, n_segments):
    return _run(x, wfilt, conv_w, conv_b, n_segments)[0]


# revision 22
# speedup vs baseline: 1.0289x; 1.0289x over previous
"""Trainium2 Bass kernel for nn_DCTiDCTWrapper3D.

Math: out = x + gelu(conv1x1(irfft(rfft(x, ch-axis) * Wc, ch-axis)) + b)

The rfft -> complex-filter -> irfft chain along the 1024-channel axis is a
linear map, implemented as two dense 1024x1024 matmuls with a packed real
spectrum of exactly 1024 slots (513 Re + 511 Im; the imaginary parts of bins
0 and 512 never reach the irfft output). The irfft basis is folded into the
1x1 conv weights host-side (M2 = Gp @ conv_w.T), so the device computes:

  per (b, t):  XF^T = M1^T @ x          [1024 slots, 392 pos]  (PE)
               Y_c  = XF_c*A_c + XF_{(c+4)%8}*B_c              (DVE)
               e^T  = gelu(M2^T @ Y + b) [1024 cout, 392 pos]  (PE+ACT)

and ships e back in bf16; the residual out = x + e happens on the host
(the residual path involves no device compute and skipping it removes
25 MB/core of fp32 DMA).

Device pipeline notes (from perfetto trace analysis of v1):
- The PE streams fp8 DoubleRow matmuls at ~163ns each (157 TF/s) with
  LDWEIGHTS fully hidden, so 512 matmuls/core = ~84us is the PE floor.
  Everything else is sized to hide underneath it.
- DVE fast modes need all operands 2-byte, packed, in SBUF: the spectral
  filter A/B are pre-tiled per position host-side (b-duplicated) so the
  muls are plain elementwise bf16, no stride-0 broadcast.
- PSUM->SBUF X copies are pair-fused on ACT: stage-1 writes chunk pairs
  (c, c+4) into one 2-bank PSUM tile, one ACT copy moves both (the two
  chunks a filter pair needs), halving ACT's ~185ns fixed per-op cost.
- One contiguous DMA per tensor per t (128 descriptors of 3-6KB each vs
  53k small descriptors in v1).

Sharding: data-parallel over the batch dim b (16 clips / 8 cores = 2 per
core); filter + DFT/conv weights replicated.
"""

import os
import sys

import numpy as np

for _p in ("/opt/trn_rl_repo", "/root/.axon_site/_ro/trn_rl_repo"):
    if os.path.isdir(_p) and _p not in sys.path:
        sys.path.append(_p)

import ml_dtypes

import concourse.bass as bass
import concourse.mybir as mybir
import concourse.tile as tile
from concourse import bacc
from concourse.bass_utils import run_bass_kernel_spmd

B, T, C, H, W = 16, 8, 1024, 14, 14
HW = H * W            # 196
NCORES = 8
BPC = B // NCORES     # 2 samples per core
KCH = C // 128        # 8 channel/slot chunks
NPOS = BPC * HW       # 392 matmul free dim
NPAD = 400            # fp8 DoubleRow interleave stride (16B aligned)
F32 = mybir.dt.float32
BF16 = mybir.dt.bfloat16
FP8 = mybir.dt.float8e4
NP_FP8 = ml_dtypes.float8_e4m3
NP_BF16 = ml_dtypes.bfloat16
DR = mybir.MatmulPerfMode.DoubleRow
PF = 2                # DMA prefetch depth in t iterations


def _dft_matrices():
    """Packed rfft matrix M1 [c, slot] and packed irfft basis Gp [slot, c],
    both ortho-normalized, slot layout: 0..512 Re(bin), 512+k Im(bin k)."""
    n = np.arange(C, dtype=np.float64)
    k_re = np.arange(513, dtype=np.float64)
    k_im = np.arange(1, 512, dtype=np.float64)
    s = 1.0 / np.sqrt(C)
    M1 = np.empty((C, C), np.float64)
    M1[:, :513] = np.cos(2 * np.pi * np.outer(n, k_re) / C) * s
    M1[:, 513:] = -np.sin(2 * np.pi * np.outer(n, k_im) / C) * s
    Gp = np.empty((C, C), np.float64)
    Gp[0, :] = s
    Gp[1:512, :] = 2.0 * np.cos(2 * np.pi * np.outer(k_im, n) / C) * s
    Gp[512, :] = ((-1.0) ** n) * s
    Gp[513:, :] = -2.0 * np.sin(2 * np.pi * np.outer(k_im, n) / C) * s
    return M1, Gp


def _pack_weights_fp8(Mx):
    """[c, slot] -> [128, mf, g, i, 128] fp8 flat [128, 8192] for DoubleRow
    lhsT tiles (c = g*256 + i*128 + p), output-chunk-major so the weight
    load can be split into per-chunk DMAs at startup."""
    w = Mx.astype(np.float32).reshape(4, 2, 128, KCH, 128)  # g i p mf c
    w = w.transpose(2, 3, 0, 1, 4).reshape(128, 4 * 2 * KCH * 128)
    return np.ascontiguousarray(w).astype(NP_FP8)


def _build_nc():
    nc = bacc.Bacc(
        "TRN2", target_bir_lowering=False, debug=False, num_devices=NCORES
    )
    x8_d = nc.dram_tensor("x8", [T, 128, 4, 2, NPAD], FP8, kind="ExternalInput").ap()
    m1_d = nc.dram_tensor("m1", [128, KCH, 4 * 2 * 128], FP8, kind="ExternalInput").ap()
    m2_d = nc.dram_tensor("m2", [128, KCH * C], FP8, kind="ExternalInput").ap()
    ab_d = nc.dram_tensor("ab", [T, 128, 2, KCH, NPOS], BF16, kind="ExternalInput").ap()
    abs_d = nc.dram_tensor("abs", [2, 128, 2, KCH, HW], BF16, kind="ExternalInput").ap()
    bias_d = nc.dram_tensor("bias", [128, KCH], F32, kind="ExternalInput").ap()
    out_d = nc.dram_tensor("out", [T, 128, KCH, NPOS], BF16, kind="ExternalOutput").ap()

    from contextlib import ExitStack

    with tile.TileContext(nc) as tc, ExitStack() as ctx:
        const = ctx.enter_context(tc.tile_pool(name="const", bufs=1))
        x8_pool = ctx.enter_context(tc.tile_pool(name="x8", bufs=PF + 1))
        ab_pool = ctx.enter_context(tc.tile_pool(name="ab", bufs=PF + 1))
        xb_pool = ctx.enter_context(tc.tile_pool(name="xb", bufs=2))
        y_pool = ctx.enter_context(tc.tile_pool(name="y", bufs=2))
        tmp_pool = ctx.enter_context(tc.tile_pool(name="tmp", bufs=4))
        o_pool = ctx.enter_context(tc.tile_pool(name="o", bufs=3))
        ps1_pool = ctx.enter_context(tc.tile_pool(name="ps1", bufs=2, space="PSUM"))
        ps2_pool = ctx.enter_context(tc.tile_pool(name="ps2", bufs=3, space="PSUM"))

        m1_sb = const.tile([128, KCH, 4, 2, 128], FP8)
        m2_sb = const.tile([128, KCH * C], FP8)
        bias_sb = const.tile([128, KCH], F32)
        m2_t = m2_sb[:, :].rearrange("p (m g i c) -> p m g i c", g=4, i=2, m=KCH)

        x8_tiles, ab_tiles = {}, {}

        def issue_dma(t):
            x8t = x8_pool.tile([128, 4, 2, NPAD], FP8, tag="x8")
            nc.sync.dma_start(x8t[:, :, :, :], x8_d[t])
            ab_sb = ab_pool.tile([128, 2, KCH, NPOS], BF16, tag="ab")
            nc.sync.dma_start(ab_sb[:, :, :, :], ab_d[t])
            x8_tiles[t], ab_tiles[t] = x8t, ab_sb

        # startup order: what the first stage-1 pair needs lands first —
        # x8 of t0 and the m1 chunks for pair 0 (m1 is packed host-side in
        # pair order 0,4,1,5,2,6,3,7 so this is a 2-chunk prefix). t0/t1
        # use compact untiled filters (broadcast muls) to halve the
        # startup DMA burst; steady-state t>=2 use the pre-tiled layout.
        x80 = x8_pool.tile([128, 4, 2, NPAD], FP8, tag="x8")
        nc.sync.dma_start(x80[:, :, :, :], x8_d[0])
        nc.sync.dma_start(m1_sb[:, 0:2], m1_d[:, 0:2])
        ab0 = ab_pool.tile([128, 2, KCH, HW], BF16, tag="abs")
        nc.sync.dma_start(ab0[:, :, :, :], abs_d[0])
        nc.sync.dma_start(m1_sb[:, 2:8], m1_d[:, 2:8])
        x8_tiles[0], ab_tiles[0] = x80, ab0
        x81 = x8_pool.tile([128, 4, 2, NPAD], FP8, tag="x8")
        nc.sync.dma_start(x81[:, :, :, :], x8_d[1])
        nc.sync.dma_start(m2_sb[:, :], m2_d[:, :])
        nc.sync.dma_start(bias_sb[:, :], bias_d[:, :])
        ab1 = ab_pool.tile([128, 2, KCH, HW], BF16, tag="abs")
        nc.sync.dma_start(ab1[:, :, :, :], abs_d[1])
        x8_tiles[1], ab_tiles[1] = x81, ab1

        def stage2(t, y_sb):
            o_t = o_pool.tile([128, KCH, NPOS], BF16, tag="o")
            for mo in range(KCH):
                ps = ps2_pool.tile([128, NPOS], F32, tag="ps2")
                for g in range(4):
                    nc.tensor.matmul(
                        ps[:, :],
                        m2_t[:, mo, g, :, :],
                        y_sb[:, g, :, :NPOS],
                        start=(g == 0),
                        stop=(g == 3),
                        perf_mode=DR,
                    )
                nc.scalar.activation(
                    o_t[:, mo, :],
                    ps[:, :],
                    mybir.ActivationFunctionType.Gelu,
                    bias=bias_sb[:, mo : mo + 1],
                )
                # drain the first half early so the tail t's output DMA
                # overlaps the second half's compute
                if mo == 3:
                    nc.sync.dma_start(out_d[t][:, :4, :], o_t[:, :4, :])
            nc.sync.dma_start(out_d[t][:, 4:, :], o_t[:, 4:, :])

        pending = None
        for t in range(T):
            if t + PF < T:
                issue_dma(t + PF)
            x8t, ab_sb = x8_tiles.pop(t), ab_tiles.pop(t)

            # stage 1: X = M1^T @ x, chunk pairs (c, c+4) share a 2-bank
            # PSUM tile so one ACT copy drains both
            xb_t = xb_pool.tile([128, 4, 2, NPOS], BF16, tag="xb")
            y_sb = y_pool.tile([128, 4, 2, NPAD], FP8, tag="y")
            for p in range(4):
                ps = ps1_pool.tile([128, 2, 512], F32, tag="ps1")
                for half, c in ((0, p), (1, p + 4)):
                    for g in range(4):
                        nc.tensor.matmul(
                            ps[:, half, :NPOS],
                            m1_sb[:, 2 * (c % 4) + c // 4, g, :, :],
                            x8t[:, g, :, :NPOS],
                            start=(g == 0),
                            stop=(g == 3),
                            perf_mode=DR,
                        )
                nc.scalar.copy(xb_t[:, p, :, :], ps[:, :, :NPOS])
                # spectral filter: Y_c = A_c*X_c + B_c*X_{c+4} (and mirror)
                for half, c in ((0, p), (1, p + 4)):
                    p_t = tmp_pool.tile([128, NPOS], BF16, tag="p")
                    q_t = tmp_pool.tile([128, NPOS], BF16, tag="q")
                    if t < 2:  # untiled filter: broadcast over the 2 samples
                        xv = xb_t[:, p, half, :].rearrange("p (b s) -> p b s", b=BPC)
                        xs = xb_t[:, p, 1 - half, :].rearrange("p (b s) -> p b s", b=BPC)
                        pv = p_t[:, :].rearrange("p (b s) -> p b s", b=BPC)
                        qv = q_t[:, :].rearrange("p (b s) -> p b s", b=BPC)
                        nc.vector.tensor_mul(
                            pv, *bass.broadcast_tensor_aps(xv, ab_sb[:, 0, c : c + 1, :])
                        )
                        nc.vector.tensor_mul(
                            qv, *bass.broadcast_tensor_aps(xs, ab_sb[:, 1, c : c + 1, :])
                        )
                    else:
                        nc.vector.tensor_mul(p_t[:, :], xb_t[:, p, half, :], ab_sb[:, 0, c, :])
                        nc.vector.tensor_mul(q_t[:, :], xb_t[:, p, 1 - half, :], ab_sb[:, 1, c, :])
                    nc.vector.tensor_add(y_sb[:, c // 2, c % 2, :NPOS], p_t[:, :], q_t[:, :])

            # stage 2 runs one t behind stage 1 so the PE stream
            # S1(t) -> S2(t-1) -> S1(t+1) never waits on the filter chain
            if pending is not None:
                stage2(*pending)
            pending = (t, y_sb)
        stage2(*pending)

    nc.compile()
    return nc


_CACHE = {}


def _get_compiled():
    if "nc" not in _CACHE:
        _CACHE["nc"] = _build_nc()
    return _CACHE["nc"]


def _host_prep(wfilt, conv_w, conv_b):
    M1, Gp = _dft_matrices()
    M2 = Gp @ conv_w.astype(np.float64).T
    m1_np = _pack_weights_fp8(M1)
    m2_np = _pack_weights_fp8(M2)

    # A/B filter tensors: [t, slot, hw], slots packed as in _dft_matrices
    Wr = wfilt[..., 0].reshape(T, HW, 513).transpose(0, 2, 1)  # [t, k, hw]
    Wi = wfilt[..., 1].reshape(T, HW, 513).transpose(0, 2, 1)
    A = np.zeros((T, C, HW), np.float32)
    Bf = np.zeros((T, C, HW), np.float32)
    A[:, :513] = Wr
    A[:, 513:] = Wr[:, 1:512]
    Bf[:, 1:512] = -Wi[:, 1:512]
    Bf[:, 513:] = Wi[:, 1:512]
    # chunk to [t, 128, k, hw], duplicate positions for the BPC=2 samples,
    # and stack A/B into one tensor (one DMA per t)
    A = A.reshape(T, KCH, 128, HW).transpose(0, 2, 1, 3)
    Bf = Bf.reshape(T, KCH, 128, HW).transpose(0, 2, 1, 3)
    ab = np.stack([A, Bf], axis=1).transpose(0, 2, 1, 3, 4)  # [t, 128, 2, k, hw]
    abs_np = np.ascontiguousarray(ab[:2]).astype(NP_BF16)
    ab_np = np.ascontiguousarray(np.tile(ab, (1, 1, 1, 1, BPC))).astype(NP_BF16)
    # [128, KCH, g*i*128] with chunks in filter-pair order (0,4,1,5,...) so
    # the first startup DMA carries exactly the two chunks pair 0 needs
    m1_np = np.ascontiguousarray(
        m1_np.reshape(128, KCH, 1024)[:, [0, 4, 1, 5, 2, 6, 3, 7], :]
    )
    bias_np = np.ascontiguousarray(conv_b.reshape(KCH, 128).T.astype(np.float32))
    return m1_np, m2_np, ab_np, abs_np, bias_np


def _run(x, wfilt, conv_w, conv_b, n_segments, **spmd_kwargs):
    assert int(n_segments) == T and x.shape == (B * T, C, H, W)
    x = np.ascontiguousarray(x, dtype=np.float32)
    m1_np, m2_np, ab_np, abs_np, bias_np = _host_prep(
        np.asarray(wfilt, np.float32),
        np.asarray(conv_w, np.float32),
        np.asarray(conv_b, np.float32),
    )
    # x [bt, c, h, w] -> fp8 [core, t, 128, g, i, NPAD] with both local
    # samples adjacent on the position axis (b-major within a t)
    x8 = x.reshape(NCORES, BPC, T, 4, 2, 128, HW).astype(NP_FP8)
    x8 = x8.transpose(0, 2, 5, 3, 4, 1, 6)  # core t p g i b s
    x8p = np.zeros((NCORES, T, 128, 4, 2, NPAD), NP_FP8)
    x8p[..., :NPOS] = x8.reshape(NCORES, T, 128, 4, 2, NPOS)

    nc = _get_compiled()
    in_maps = [
        {
            "x8": x8p[i],
            "m1": m1_np,
            "m2": m2_np,
            "ab": ab_np,
            "abs": abs_np,
            "bias": bias_np,
        }
        for i in range(NCORES)
    ]
    res = run_bass_kernel_spmd(nc, in_maps, list(range(NCORES)), **spmd_kwargs)
    e = np.stack([r["out"] for r in res.results])  # [core, t, 128, k, 392]
    e = e.reshape(NCORES, T, 128, KCH, BPC, HW).astype(np.float32)
    e = e.transpose(0, 4, 1, 3, 2, 5).reshape(B * T, C, H, W)
    full = x + e
    return full, res


def kernel(x, wfilt, conv_w, conv_b, n_segments):
    return _run(x, wfilt, conv_w, conv_b, n_segments)[0]


# revision 27
# speedup vs baseline: 1.0561x; 1.0264x over previous
"""Trainium2 Bass kernel for nn_DCTiDCTWrapper3D.

Math: out = x + gelu(conv1x1(irfft(rfft(x, ch-axis) * Wc, ch-axis)) + b)

The rfft -> complex-filter -> irfft chain along the 1024-channel axis is a
linear map, implemented as two dense 1024x1024 matmuls with a packed real
spectrum of exactly 1024 slots (513 Re + 511 Im; the imaginary parts of bins
0 and 512 never reach the irfft output). The irfft basis is folded into the
1x1 conv weights host-side (M2 = Gp @ conv_w.T), so the device computes:

  per (b, t):  XF^T = M1^T @ x          [1024 slots, 392 pos]  (PE)
               Y_c  = XF_c*A_c + XF_{(c+4)%8}*B_c              (DVE)
               e^T  = gelu(M2^T @ Y + b) [1024 cout, 392 pos]  (PE+ACT)

and ships e back in bf16; the residual out = x + e happens on the host
(the residual path involves no device compute and skipping it removes
25 MB/core of fp32 DMA).

Device pipeline notes (from perfetto trace analysis of v1):
- The PE streams fp8 DoubleRow matmuls at ~163ns each (157 TF/s) with
  LDWEIGHTS fully hidden, so 512 matmuls/core = ~84us is the PE floor.
  Everything else is sized to hide underneath it.
- DVE fast modes need all operands 2-byte, packed, in SBUF: the spectral
  filter A/B are pre-tiled per position host-side (b-duplicated) so the
  muls are plain elementwise bf16, no stride-0 broadcast.
- PSUM->SBUF X copies are pair-fused on ACT: stage-1 writes chunk pairs
  (c, c+4) into one 2-bank PSUM tile, one ACT copy moves both (the two
  chunks a filter pair needs), halving ACT's ~185ns fixed per-op cost.
- One contiguous DMA per tensor per t (128 descriptors of 3-6KB each vs
  53k small descriptors in v1).

Sharding: data-parallel over the batch dim b (16 clips / 8 cores = 2 per
core); filter + DFT/conv weights replicated.
"""

import os
import sys

import numpy as np

for _p in ("/opt/trn_rl_repo", "/root/.axon_site/_ro/trn_rl_repo"):
    if os.path.isdir(_p) and _p not in sys.path:
        sys.path.append(_p)

import ml_dtypes

import concourse.bass as bass
import concourse.mybir as mybir
import concourse.tile as tile
from concourse import bacc
from concourse.bass_utils import run_bass_kernel_spmd

B, T, C, H, W = 16, 8, 1024, 14, 14
HW = H * W            # 196
NCORES = 8
BPC = B // NCORES     # 2 samples per core
KCH = C // 128        # 8 channel/slot chunks
NPOS = BPC * HW       # 392 matmul free dim
NPAD = 400            # fp8 DoubleRow interleave stride (16B aligned)
F32 = mybir.dt.float32
BF16 = mybir.dt.bfloat16
FP8 = mybir.dt.float8e4
NP_FP8 = ml_dtypes.float8_e4m3
NP_BF16 = ml_dtypes.bfloat16
DR = mybir.MatmulPerfMode.DoubleRow
PF = 2                # DMA prefetch depth in t iterations


def _dft_matrices():
    """Packed rfft matrix M1 [c, slot] and packed irfft basis Gp [slot, c],
    both ortho-normalized, slot layout: 0..512 Re(bin), 512+k Im(bin k)."""
    n = np.arange(C, dtype=np.float64)
    k_re = np.arange(513, dtype=np.float64)
    k_im = np.arange(1, 512, dtype=np.float64)
    s = 1.0 / np.sqrt(C)
    M1 = np.empty((C, C), np.float64)
    M1[:, :513] = np.cos(2 * np.pi * np.outer(n, k_re) / C) * s
    M1[:, 513:] = -np.sin(2 * np.pi * np.outer(n, k_im) / C) * s
    Gp = np.empty((C, C), np.float64)
    Gp[0, :] = s
    Gp[1:512, :] = 2.0 * np.cos(2 * np.pi * np.outer(k_im, n) / C) * s
    Gp[512, :] = ((-1.0) ** n) * s
    Gp[513:, :] = -2.0 * np.sin(2 * np.pi * np.outer(k_im, n) / C) * s
    return M1, Gp


def _pack_weights_fp8(Mx):
    """[c, slot] -> [128, mf, g, i, 128] fp8 flat [128, 8192] for DoubleRow
    lhsT tiles (c = g*256 + i*128 + p), output-chunk-major so the weight
    load can be split into per-chunk DMAs at startup."""
    w = Mx.astype(np.float32).reshape(4, 2, 128, KCH, 128)  # g i p mf c
    w = w.transpose(2, 3, 0, 1, 4).reshape(128, 4 * 2 * KCH * 128)
    return np.ascontiguousarray(w).astype(NP_FP8)


def _build_nc():
    nc = bacc.Bacc(
        "TRN2", target_bir_lowering=False, debug=False, num_devices=NCORES
    )
    x8_d = nc.dram_tensor("x8", [T, 128, 4, 2, NPAD], FP8, kind="ExternalInput").ap()
    m1_d = nc.dram_tensor("m1", [128, KCH, 4 * 2 * 128], FP8, kind="ExternalInput").ap()
    m2_d = nc.dram_tensor("m2", [128, KCH * C], FP8, kind="ExternalInput").ap()
    ab_d = nc.dram_tensor("ab", [T, 128, 2, KCH, NPOS], BF16, kind="ExternalInput").ap()
    abs_d = nc.dram_tensor("abs", [2, 128, 2, KCH, HW], BF16, kind="ExternalInput").ap()
    bias_d = nc.dram_tensor("bias", [128, KCH], F32, kind="ExternalInput").ap()
    out_d = nc.dram_tensor("out", [T, 128, KCH, NPOS], BF16, kind="ExternalOutput").ap()

    from contextlib import ExitStack

    with tile.TileContext(nc) as tc, ExitStack() as ctx:
        const = ctx.enter_context(tc.tile_pool(name="const", bufs=1))
        x8_pool = ctx.enter_context(tc.tile_pool(name="x8", bufs=PF + 1))
        ab_pool = ctx.enter_context(tc.tile_pool(name="ab", bufs=PF + 1))
        xb_pool = ctx.enter_context(tc.tile_pool(name="xb", bufs=2))
        y_pool = ctx.enter_context(tc.tile_pool(name="y", bufs=2))
        tmp_pool = ctx.enter_context(tc.tile_pool(name="tmp", bufs=4))
        o_pool = ctx.enter_context(tc.tile_pool(name="o", bufs=3))
        ps1_pool = ctx.enter_context(tc.tile_pool(name="ps1", bufs=2, space="PSUM"))
        ps2_pool = ctx.enter_context(tc.tile_pool(name="ps2", bufs=3, space="PSUM"))

        m1_sb = const.tile([128, KCH, 4, 2, 128], FP8)
        m2_sb = const.tile([128, KCH * C], FP8)
        bias_sb = const.tile([128, KCH], F32)
        m2_t = m2_sb[:, :].rearrange("p (m g i c) -> p m g i c", g=4, i=2, m=KCH)

        x8_tiles, ab_tiles = {}, {}

        def issue_dma(t):
            x8t = x8_pool.tile([128, 4, 2, NPAD], FP8, tag="x8")
            nc.sync.dma_start(x8t[:, :, :, :], x8_d[t])
            ab_sb = ab_pool.tile([128, 2, KCH, NPOS], BF16, tag="ab")
            nc.sync.dma_start(ab_sb[:, :, :, :], ab_d[t])
            x8_tiles[t], ab_tiles[t] = x8t, ab_sb

        # startup order: what the first stage-1 pair needs lands first —
        # x8 of t0 and the m1 chunks for pair 0 (m1 is packed host-side in
        # pair order 0,4,1,5,2,6,3,7 so this is a 2-chunk prefix). t0/t1
        # use compact untiled filters (broadcast muls) to halve the
        # startup DMA burst; steady-state t>=2 use the pre-tiled layout.
        x80 = x8_pool.tile([128, 4, 2, NPAD], FP8, tag="x8")
        nc.sync.dma_start(x80[:, :, :, :], x8_d[0])
        nc.sync.dma_start(m1_sb[:, 0:2], m1_d[:, 0:2])
        nc.sync.dma_start(m1_sb[:, 2:8], m1_d[:, 2:8])
        ab0 = ab_pool.tile([128, 2, KCH, HW], BF16, tag="abs")
        nc.sync.dma_start(ab0[:, :, :, :], abs_d[0])
        x8_tiles[0], ab_tiles[0] = x80, ab0
        x81 = x8_pool.tile([128, 4, 2, NPAD], FP8, tag="x8")
        nc.sync.dma_start(x81[:, :, :, :], x8_d[1])
        nc.sync.dma_start(m2_sb[:, :], m2_d[:, :])
        nc.sync.dma_start(bias_sb[:, :], bias_d[:, :])
        ab1 = ab_pool.tile([128, 2, KCH, HW], BF16, tag="abs")
        nc.sync.dma_start(ab1[:, :, :, :], abs_d[1])
        x8_tiles[1], ab_tiles[1] = x81, ab1

        def stage2_mo(t, y_sb, o_t, mo):
            ps = ps2_pool.tile([128, NPOS], F32, tag="ps2")
            for g in range(4):
                nc.tensor.matmul(
                    ps[:, :],
                    m2_t[:, mo, g, :, :],
                    y_sb[:, g, :, :NPOS],
                    start=(g == 0),
                    stop=(g == 3),
                    perf_mode=DR,
                )
            nc.scalar.activation(
                o_t[:, mo, :],
                ps[:, :],
                mybir.ActivationFunctionType.Gelu,
                bias=bias_sb[:, mo : mo + 1],
            )
            # drain each half as soon as it is complete so the output DMA
            # overlaps the remaining compute
            if mo == 3:
                nc.sync.dma_start(out_d[t][:, :4, :], o_t[:, :4, :])
            elif mo == 7:
                nc.sync.dma_start(out_d[t][:, 4:, :], o_t[:, 4:, :])

        pending = None
        for t in range(T):
            if t + PF < T:
                issue_dma(t + PF)
            x8t, ab_sb = x8_tiles.pop(t), ab_tiles.pop(t)

            # stage 1: X = M1^T @ x, chunk pairs (c, c+4) share a 2-bank
            # PSUM tile so one ACT copy drains both. Stage 2 of t-1 is
            # interleaved two output chunks per stage-1 pair, so the PE
            # stream and the ACT copy/gelu load stay evenly spread.
            xb_t = xb_pool.tile([128, 4, 2, NPOS], BF16, tag="xb")
            y_sb = y_pool.tile([128, 4, 2, NPAD], FP8, tag="y")
            o_prev = None
            if pending is not None:
                o_prev = o_pool.tile([128, KCH, NPOS], BF16, tag="o")
            for p in range(4):
                ps = ps1_pool.tile([128, 2, 512], F32, tag="ps1")
                for half, c in ((0, p), (1, p + 4)):
                    for g in range(4):
                        nc.tensor.matmul(
                            ps[:, half, :NPOS],
                            m1_sb[:, 2 * (c % 4) + c // 4, g, :, :],
                            x8t[:, g, :, :NPOS],
                            start=(g == 0),
                            stop=(g == 3),
                            perf_mode=DR,
                        )
                nc.scalar.copy(xb_t[:, p, :, :], ps[:, :, :NPOS])
                # spectral filter: Y_c = A_c*X_c + B_c*X_{c+4} (and mirror)
                for half, c in ((0, p), (1, p + 4)):
                    p_t = tmp_pool.tile([128, NPOS], BF16, tag="p")
                    q_t = tmp_pool.tile([128, NPOS], BF16, tag="q")
                    if t < 2:  # untiled filter: broadcast over the 2 samples
                        xv = xb_t[:, p, half, :].rearrange("p (b s) -> p b s", b=BPC)
                        xs = xb_t[:, p, 1 - half, :].rearrange("p (b s) -> p b s", b=BPC)
                        pv = p_t[:, :].rearrange("p (b s) -> p b s", b=BPC)
                        qv = q_t[:, :].rearrange("p (b s) -> p b s", b=BPC)
                        nc.vector.tensor_mul(
                            pv, *bass.broadcast_tensor_aps(xv, ab_sb[:, 0, c : c + 1, :])
                        )
                        nc.vector.tensor_mul(
                            qv, *bass.broadcast_tensor_aps(xs, ab_sb[:, 1, c : c + 1, :])
                        )
                    else:
                        nc.vector.tensor_mul(p_t[:, :], xb_t[:, p, half, :], ab_sb[:, 0, c, :])
                        nc.vector.tensor_mul(q_t[:, :], xb_t[:, p, 1 - half, :], ab_sb[:, 1, c, :])
                    nc.vector.tensor_add(y_sb[:, c // 2, c % 2, :NPOS], p_t[:, :], q_t[:, :])
                if pending is not None:
                    stage2_mo(*pending, o_prev, 2 * p)
                    stage2_mo(*pending, o_prev, 2 * p + 1)
            pending = (t, y_sb)
        o_last = o_pool.tile([128, KCH, NPOS], BF16, tag="o")
        for mo in range(KCH):
            stage2_mo(*pending, o_last, mo)

    nc.compile()
    return nc


_CACHE = {}


def _get_compiled():
    if "nc" not in _CACHE:
        _CACHE["nc"] = _build_nc()
    return _CACHE["nc"]


def _host_prep(wfilt, conv_w, conv_b):
    M1, Gp = _dft_matrices()
    M2 = Gp @ conv_w.astype(np.float64).T
    m1_np = _pack_weights_fp8(M1)
    m2_np = _pack_weights_fp8(M2)

    # A/B filter tensors: [t, slot, hw], slots packed as in _dft_matrices
    Wr = wfilt[..., 0].reshape(T, HW, 513).transpose(0, 2, 1)  # [t, k, hw]
    Wi = wfilt[..., 1].reshape(T, HW, 513).transpose(0, 2, 1)
    A = np.zeros((T, C, HW), np.float32)
    Bf = np.zeros((T, C, HW), np.float32)
    A[:, :513] = Wr
    A[:, 513:] = Wr[:, 1:512]
    Bf[:, 1:512] = -Wi[:, 1:512]
    Bf[:, 513:] = Wi[:, 1:512]
    # chunk to [t, 128, k, hw], duplicate positions for the BPC=2 samples,
    # and stack A/B into one tensor (one DMA per t)
    A = A.reshape(T, KCH, 128, HW).transpose(0, 2, 1, 3)
    Bf = Bf.reshape(T, KCH, 128, HW).transpose(0, 2, 1, 3)
    ab = np.stack([A, Bf], axis=1).transpose(0, 2, 1, 3, 4)  # [t, 128, 2, k, hw]
    abs_np = np.ascontiguousarray(ab[:2]).astype(NP_BF16)
    ab_np = np.ascontiguousarray(np.tile(ab, (1, 1, 1, 1, BPC))).astype(NP_BF16)
    # [128, KCH, g*i*128] with chunks in filter-pair order (0,4,1,5,...) so
    # the first startup DMA carries exactly the two chunks pair 0 needs
    m1_np = np.ascontiguousarray(
        m1_np.reshape(128, KCH, 1024)[:, [0, 4, 1, 5, 2, 6, 3, 7], :]
    )
    bias_np = np.ascontiguousarray(conv_b.reshape(KCH, 128).T.astype(np.float32))
    return m1_np, m2_np, ab_np, abs_np, bias_np


def _run(x, wfilt, conv_w, conv_b, n_segments, **spmd_kwargs):
    assert int(n_segments) == T and x.shape == (B * T, C, H, W)
    x = np.ascontiguousarray(x, dtype=np.float32)
    m1_np, m2_np, ab_np, abs_np, bias_np = _host_prep(
        np.asarray(wfilt, np.float32),
        np.asarray(conv_w, np.float32),
        np.asarray(conv_b, np.float32),
    )
    # x [bt, c, h, w] -> fp8 [core, t, 128, g, i, NPAD] with both local
    # samples adjacent on the position axis (b-major within a t)
    x8 = x.reshape(NCORES, BPC, T, 4, 2, 128, HW).astype(NP_FP8)
    x8 = x8.transpose(0, 2, 5, 3, 4, 1, 6)  # core t p g i b s
    x8p = np.zeros((NCORES, T, 128, 4, 2, NPAD), NP_FP8)
    x8p[..., :NPOS] = x8.reshape(NCORES, T, 128, 4, 2, NPOS)

    nc = _get_compiled()
    in_maps = [
        {
            "x8": x8p[i],
            "m1": m1_np,
            "m2": m2_np,
            "ab": ab_np,
            "abs": abs_np,
            "bias": bias_np,
        }
        for i in range(NCORES)
    ]
    res = run_bass_kernel_spmd(nc, in_maps, list(range(NCORES)), **spmd_kwargs)
    e = np.stack([r["out"] for r in res.results])  # [core, t, 128, k, 392]
    e = e.reshape(NCORES, T, 128, KCH, BPC, HW).astype(np.float32)
    e = e.transpose(0, 4, 1, 3, 2, 5).reshape(B * T, C, H, W)
    full = x + e
    return full, res


def kernel(x, wfilt, conv_w, conv_b, n_segments):
    return _run(x, wfilt, conv_w, conv_b, n_segments)[0]
